# revision 18
# baseline (speedup 1.0000x reference)
"""Causal attention product kernel for Trainium2, SPMD over 8 NeuronCores.

Math (faithful to the nn.Module reference):
    scores = (Q @ K^T) / 8 + mask          [B,H,S,S], mask is [B,1,1,S]
    scores[..., -128:, -128:] = tril(ones,-1).T * finfo.min   (overwrite!)
    out = softmax(scores, -1) @ V

Sharding: B*H = 24 heads split 3-per-core across 8 cores; no cross-core
communication.

Per-core algorithm (per head), flash-attention style -- the [S,S] score
matrix never hits DRAM:
  - Host pre-transposes Q,K to [64, S] fp16 and pre-scales V rows by
    exp(mask_k) (with an appended exp(mask_k) ones-column that accumulates
    the softmax denominator).  exp(s + m) = exp(s)*exp(m) makes the
    additive mask exact while keeping the matmul contraction at d=64.
  - For each 512-query block: S^T tiles [128k, 512q] on PE (fp16 in, fp32
    psum), exp on ACT (psum -> fp16 sbuf, the 1/8 scale fused into the
    activation), PV matmul with V-stationary [128, 65] fp16 weights
    accumulating OUT^T [65, 512] in PSUM.  PE-transpose back to [q, d],
    divide by the denominator on DVE, DMA out.
  - The overwritten bottom-right 128x128 block of probs is exactly
    tril(ones) * exp(-mask_k) (so the V pre-scale cancels): DMA'd from the
    host straight over P^T before the PV matmul.
"""

import os
import sys

for _p in ("/opt/trn_rl_repo", "/root/.axon_site/_ro/trn_rl_repo"):
    if os.path.isdir(_p) and _p not in sys.path:
        sys.path.insert(0, _p)

import numpy as np

import concourse.bass as bass
import concourse.mybir as mybir
import concourse.tile as tile
from concourse import bacc
from concourse import bass_utils

B, H, S, D = 2, 12, 4096, 64
N_CORES = 8
HPC = (B * H) // N_CORES  # heads per core = 3

KTILES = S // 128  # 32 k-tiles of 128
QBS = 512          # queries per block
QB = S // QBS      # 8 query blocks
CH = 3             # k-tiles per ACT chunk (3 psum banks per S^T tile)

F32R = mybir.dt.float32r
F32 = mybir.dt.float32
F16 = mybir.dt.float16


def _chunks():
    out = []
    kt = 0
    while kt < KTILES:
        n = min(CH, KTILES - kt)
        out.append((kt, n))
        kt += n
    return out


def _kernel_body(tc, q_d, k_d, v_d, ut_d, ident_d, o_d):
    nc = tc.nc

    singles = tc.alloc_tile_pool(name="singles", bufs=1)
    qkpool = tc.alloc_tile_pool(name="qk", bufs=2)
    vpool = tc.alloc_tile_pool(name="v", bufs=2)
    ptpool = tc.alloc_tile_pool(name="pt", bufs=2)
    otpool = tc.alloc_tile_pool(name="ot", bufs=2)
    outpool = tc.alloc_tile_pool(name="outsb", bufs=3)
    rpool = tc.alloc_tile_pool(name="r", bufs=4)
    spsum = tc.alloc_tile_pool(name="spsum", bufs=2, space="PSUM")
    opsum = tc.alloc_tile_pool(name="opsum", bufs=2, space="PSUM")

    # Identity for the output PE transposes (fp32r path).
    ident = singles.tile([128, 128], F32R, name="ident")
    nc.sync.dma_start(out=ident, in_=ident_d)

    for h in range(HPC):
        # ---- load pre-transposed Q^T, K^T and pre-scaled V' ----
        qt = qkpool.tile([64, S], F16, name="qt")
        nc.sync.dma_start(out=qt, in_=q_d[h])
        ktt = qkpool.tile([64, S], F16, name="ktt")
        nc.sync.dma_start(out=ktt, in_=k_d[h])
        vt = vpool.tile([128, KTILES, D + 2], F16, name="vt")
        for g in range(8):
            nc.sync.dma_start(
                out=vt[:, g * 4 : (g + 1) * 4, :],
                in_=v_d[h, g * 512 : (g + 1) * 512, :].rearrange(
                    "(c p) f -> p c f", p=128
                ),
            )

        # ---- attention, one 512-query block at a time ----
        for qb in range(QB):
            qs = slice(qb * QBS, (qb + 1) * QBS)
            pt = ptpool.tile([128, KTILES, QBS], F16, name="pt")
            for kt0, nch in _chunks():
                sp = spsum.tile([128, CH, QBS], F32, name="sp")
                for i in range(nch):
                    kt = kt0 + i
                    nc.tensor.matmul(
                        sp[:, i, :],
                        lhsT=ktt[:, kt * 128 : (kt + 1) * 128],
                        rhs=qt[:, qs],
                        start=True,
                        stop=True,
                    )
                nc.scalar.activation(
                    out=pt[:, kt0 : kt0 + nch, :],
                    in_=sp[:, 0:nch, :],
                    func=mybir.ActivationFunctionType.Exp,
                    scale=0.125,
                )
            if qb == QB - 1:
                # overwrite probs of the bottom-right 128x128 block with the
                # host-computed tril(ones)*exp(-mask) tile
                nc.sync.dma_start(
                    out=pt[:, KTILES - 1, QBS - 128 : QBS], in_=ut_d[h]
                )

            op = opsum.tile([128, QBS], F32, name="op", tag="o")
            for kt in range(KTILES):
                nc.tensor.matmul(
                    op[0 : D + 2, :],
                    lhsT=vt[:, kt, :],
                    rhs=pt[:, kt, :],
                    start=(kt == 0),
                    stop=(kt == KTILES - 1),
                )

            # 66 rows (row 65 is V's zero pad column) so the 66-wide
            # transpose identity is a true permutation; fp32r matmuls need an
            # even innermost count.
            ot = otpool.tile([66, QBS], F32R, name="ot")
            nc.vector.tensor_copy(out=ot, in_=op[0:66, :])
            osb = outpool.tile([128, 4, D], F32, name="osb")
            tp = opsum.tile([128, QBS], F32R, name="tp", tag="o")
            for sub in range(4):
                nc.tensor.transpose(
                    tp[:, sub * 128 : sub * 128 + 66],
                    ot[:, sub * 128 : (sub + 1) * 128],
                    ident[0:66, 0:66],
                )
                r = rpool.tile([128, 1], F32, name="r")
                nc.vector.reciprocal(r, tp[:, sub * 128 + 64 : sub * 128 + 65])
                nc.vector.tensor_scalar_mul(
                    osb[:, sub, :], tp[:, sub * 128 : sub * 128 + 64], r[:, 0:1]
                )
            nc.sync.dma_start(
                out=o_d[h, qs, :].rearrange("(s p) d -> p s d", p=128), in_=osb
            )

    for pool in (opsum, spsum, rpool, outpool, otpool, ptpool, vpool, qkpool, singles):
        pool.release()


_CACHED = None


def _build():
    global _CACHED
    if _CACHED is not None:
        return _CACHED
    nc = bacc.Bacc(trn_type="TRN2", target_bir_lowering=False, debug=False)
    q_d = nc.dram_tensor("q", [HPC, D, S], F16, kind="ExternalInput").ap()
    k_d = nc.dram_tensor("k", [HPC, D, S], F16, kind="ExternalInput").ap()
    v_d = nc.dram_tensor("v", [HPC, S, D + 2], F16, kind="ExternalInput").ap()
    ut_d = nc.dram_tensor("ut", [HPC, 128, 128], F16, kind="ExternalInput").ap()
    ident_d = nc.dram_tensor("ident", [128, 128], F32R, kind="ExternalInput").ap()
    o_d = nc.dram_tensor("o", [HPC, S, D], F32, kind="ExternalOutput").ap()
    with tile.TileContext(nc) as tc:
        _kernel_body(tc, q_d, k_d, v_d, ut_d, ident_d, o_d)
    nc.compile()
    _CACHED = nc
    return nc


def _shard_inputs(query_layer, key_layer, value_layer, attention_mask):
    q = np.asarray(query_layer, dtype=np.float32).reshape(B * H, S, D)
    k = np.asarray(key_layer, dtype=np.float32).reshape(B * H, S, D)
    v = np.asarray(value_layer, dtype=np.float32).reshape(B * H, S, D)
    m = np.asarray(attention_mask, dtype=np.float32).reshape(B, S)
    m_heads = np.repeat(m, H, axis=0)  # [B*H, S]

    qt = np.ascontiguousarray(q.transpose(0, 2, 1)).astype(np.float16)
    kt = np.ascontiguousarray(k.transpose(0, 2, 1)).astype(np.float16)

    # V' = [V * exp(m_k) | exp(m_k)]; the mask rides along multiplicatively
    # and the appended column accumulates the softmax denominator.
    em = np.exp(np.clip(m_heads, -6e4, 60.0))[:, :, None]  # [B*H, S, 1]
    zc = np.zeros_like(em)
    vs = np.concatenate([v * em, em, zc], axis=2).astype(np.float16)  # [B*H,S,66]

    # P^T overwrite tile for the bottom-right block: tril(ones).T in P^T
    # layout times exp(-m) so the V' pre-scale cancels exactly.
    tri = (np.arange(128)[:, None] <= np.arange(128)[None, :]).astype(np.float32)
    inv_em = np.where(em[:, -128:, 0] > 0.0, 1.0 / np.maximum(em[:, -128:, 0], 1e-37), 0.0)
    ut = (tri[None, :, :] * inv_em[:, :, None]).astype(np.float16)  # [B*H,128,128]

    ident = np.eye(128, dtype=np.float32)

    in_maps = []
    for c in range(N_CORES):
        hs = slice(c * HPC, (c + 1) * HPC)
        in_maps.append(
            {
                "q": np.ascontiguousarray(qt[hs]),
                "k": np.ascontiguousarray(kt[hs]),
                "v": np.ascontiguousarray(vs[hs]),
                "ut": np.ascontiguousarray(ut[hs]),
                "ident": ident,
            }
        )
    return in_maps


def run(query_layer, key_layer, value_layer, attention_mask, trace=False):
    """Build + run on 8 cores; returns (full_output, BassKernelResults)."""
    nc = _build()
    in_maps = _shard_inputs(query_layer, key_layer, value_layer, attention_mask)
    res = bass_utils.run_bass_kernel_spmd(
        nc, in_maps, core_ids=list(range(N_CORES)), trace=trace
    )
    out = np.concatenate(
        [res.results[c]["o"].reshape(HPC, S, D) for c in range(N_CORES)], axis=0
    )
    return out.reshape(B, H, S, D).astype(np.float32), res


def kernel(query_layer, key_layer, value_layer, attention_mask):
    out, _ = run(query_layer, key_layer, value_layer, attention_mask)
    return out
